# revision 5
# baseline (speedup 1.0000x reference)
"""Handshaking kernel ('cat' type) for Trainium2, 8 NeuronCores.

Math: for each upper-triangular pair (i, j>=i):
    out[b, p(i,j), :] = tanh(W1 @ h_i + W2 @ h_j + bias),  W = [W1 | W2]

Decomposition: per-token projections in TRANSPOSED layout
    A_T[h, i] = (W1 @ seq^T)[h, i] + b[h],   C_T[h, j] = (W2 @ seq^T)[h, j]
so triangle row i of the pair dim is the contiguous slice
    out_T[:, OFF[i] : OFF[i]+L-i] = tanh(C_T[:, i:L] + A_T[:, i])
A_T[:, i] is a per-partition scalar: long rows run fused on ACT
(activation bias = A column, affine is free), short rows run as DVE
tensor_scalar adds (2x fp16 mode) + aligned ACT tanh slices; the split
is balanced by a measured cost model. Output is written as contiguous
fp16 chunk DMAs — no indirect scatter. Pair rows are padded to even
length so every DVE slice stays 4-byte aligned with even free dim.
Host gathers the padded columns out, transposes, and upcasts to f32.

Sharding: 8 cores = 4 batches x 2 halves of the hidden dim (H=768 ->
384 per core = 3 partition blocks of 128). All cores run the identical
program (SPMD).
"""

import sys
import numpy as np

for _p in ("/opt/trn_rl_repo", "/root/.axon_site/_ro/trn_rl_repo"):
    if _p not in sys.path:
        sys.path.insert(0, _p)

B, L, H = 4, 256, 768
HH = H // 2           # per-core hidden slice
NHB = HH // 128       # partition blocks per core (3)
NK = H // 128         # contraction blocks (6)
NPAIR = L * (L + 1) // 2      # 32896

# padded triangle layout: row i gets plen = len + (len odd), all offsets even
LENS = [L - i for i in range(L)]
PLENS = [l + (l & 1) for l in LENS]
POFF = np.concatenate([[0], np.cumsum(PLENS)]).astype(np.int64)
PTOT = int(POFF[-1])          # 33024
NCHUNK = 8
CH = PTOT // NCHUNK           # 4128
assert CH * NCHUNK == PTOT and CH % 2 == 0

# device column for each global pair index p (host-side unpad gather)
SEL = np.empty(NPAIR, np.int64)
for i in range(L):
    off = i * L - (i * (i - 1)) // 2
    SEL[off : off + LENS[i]] = POFF[i] + np.arange(LENS[i])


def build_segments():
    """Per chunk: list of (cs, ln, i, s0, odd) slices, ordered by cs.

    stage[:, cs:cs+ln] = C[:, j0:j0+ln] + A[:, i] with j0 = s0 (+1 if odd,
    read via the one-shifted C_od copy so the slice start stays even).
    """
    segs = [[] for _ in range(NCHUNK)]
    for i in range(L):
        start, end = int(POFF[i]), int(POFF[i]) + PLENS[i]
        s = start
        while s < end:
            k = s // CH
            e = min(end, (k + 1) * CH)
            j0 = i + (s - start)
            odd = j0 & 1
            segs[k].append((s - k * CH, e - s, i, j0 - odd, odd))
            s = e
    return segs


SEGS = build_segments()

# ---- static DVE/ACT load balance (measured-calibrated cost model, ns) ----
DVE_FIX, DVE_PER = 105.0, 0.52    # tensor_scalar (2x_1p fp16) per seg/elem
DVE_TT_PER = 1.04                 # merged tensor_tensor runs 1x
ACT_FIX, ACT_PER = 293.0, 0.833   # activation per instr/elem
MERGE_MAX = 100                   # merge 2-row pairs below this len


def _find_merges(dve_segs):
    """Pair adjacent equal-length full rows (even i, then i+1) into one
    merged TT op: returns (merged_pairs, leftover_singles)."""
    merged, singles = [], []
    by_key = {}
    for seg in dve_segs:
        cs, ln, i, s0, odd = seg
        by_key[(i, cs)] = seg
    used = set()
    for seg in dve_segs:
        cs, ln, i, s0, odd = seg
        if (i, cs) in used:
            continue
        # merge candidate: even row i, unsplit (cs..cs+ln is full row),
        # partner row i+1 at cs+ln with same ln, also unsplit
        if (
            ln <= MERGE_MAX
            and i % 2 == 0
            and not odd
            and ln == PLENS[i]
            and (i + 1, cs + ln) in by_key
            and (i + 1, cs + ln) not in used
        ):
            p = by_key[(i + 1, cs + ln)]
            if p[1] == ln and p[1] == PLENS[i + 1]:
                merged.append((cs, ln, i, s0))
                used.add((i, cs))
                used.add((i + 1, cs + ln))
                continue
        used.add((i, cs))
        singles.append(seg)
    return merged, singles


def plan_assignment():
    """Fuse rows with len >= thresh onto ACT; pick thresh to balance."""
    all_lens = sorted({ln for k in range(NCHUNK) for (_, ln, _, _, _) in SEGS[k]})
    best = None
    for thresh in all_lens + [1 << 30]:
        dve = act = 0.0
        for k in range(NCHUNK):
            cur = None
            nrange = 0
            dsegs = []
            for cs, ln, _i, _s0, _odd in SEGS[k]:
                if ln >= thresh:
                    act += ACT_FIX + ACT_PER * ln
                    cur = None
                else:
                    dsegs.append((cs, ln, _i, _s0, _odd))
                    if cur is None:
                        nrange += 1
                        cur = True
                    act += ACT_PER * ln
            mg, sg = _find_merges(dsegs)
            for _cs, ln, _i, _s0 in mg:
                dve += DVE_FIX + DVE_TT_PER * 2 * ln
            for _cs, ln, _i, _s0, _odd in sg:
                dve += DVE_FIX + DVE_PER * ln
            act += ACT_FIX * nrange
        dve *= NHB
        act *= NHB
        t = max(dve, act)
        if best is None or t < best[0]:
            best = (t, thresh, dve, act)
    _, thresh, dve, act = best
    plan = []
    for k in range(NCHUNK):
        fused, dves, ranges = [], [], []
        cur = None
        for seg in SEGS[k]:
            cs, ln = seg[0], seg[1]
            if ln >= thresh:
                fused.append(seg)
                cur = None
            else:
                dves.append(seg)
                if cur is None:
                    ranges.append([cs, cs + ln])
                    cur = ranges[-1]
                else:
                    cur[1] = cs + ln
        mg, sg = _find_merges(dves)
        plan.append((sg, mg, fused, ranges))
    return plan, thresh, dve, act


PLAN, THRESH, _DVE_NS, _ACT_NS = plan_assignment()


def _schedule_selfcheck():
    rng = np.random.RandomState(0)
    nh = 4
    Cc = rng.randn(nh, L).astype(np.float64)
    Aa = rng.randn(nh, L).astype(np.float64)
    Ce = np.concatenate([Cc, np.zeros((nh, 2))], axis=1)   # C_sb w/ pad
    Co = np.concatenate([Cc[:, 1:], np.zeros((nh, 1))], axis=1)  # C_od
    dev = np.full((nh, PTOT), np.nan)
    for k in range(NCHUNK):
        dves, merged, fused, ranges = PLAN[k]
        for cs, ln, i, s0, odd in dves + fused:
            src = Co if odd else Ce
            dev[:, k * CH + cs : k * CH + cs + ln] = (
                src[:, s0 : s0 + ln] + Aa[:, i : i + 1]
            )
        for cs, ln, i, s0 in merged:
            # row i from C[s0:s0+ln], row i+1 from C[s0+1:s0+1+ln]
            for v in range(2):
                dev[:, k * CH + cs + v * ln : k * CH + cs + (v + 1) * ln] = (
                    Ce[:, s0 + v : s0 + v + ln] + Aa[:, i + v : i + v + 1]
                )
        # every dve seg must fall inside an aligned range
        for cs, ln, _i, _s0, _odd in dves:
            assert any(a <= cs and cs + ln <= b for a, b in ranges)
        for cs, ln, _i, _s0 in merged:
            assert any(a <= cs and cs + 2 * ln <= b for a, b in ranges)
        # ranges must not overlap fused segs
        for cs, ln, _i, _s0, _odd in fused:
            assert all(b <= cs or cs + ln <= a for a, b in ranges)
    assert not np.isnan(dev).any()
    got = dev[:, SEL]
    ii, jj = np.triu_indices(L)
    exp = Aa[:, ii] + Cc[:, jj]
    assert np.allclose(got, exp), "segment schedule self-check failed"


_schedule_selfcheck()

_CACHE = {}


def _build_nc():
    import bass_rust
    import concourse.bass as bass
    import concourse.bacc as bacc
    import concourse.mybir as mybir
    import concourse.tile as tile

    f32 = mybir.dt.float32
    f16 = mybir.dt.float16
    Tanh = mybir.ActivationFunctionType.Tanh

    nc = bacc.Bacc(None, target_bir_lowering=False, debug=False)

    # host-packed inputs (see kernel() for layouts)
    seqT = nc.dram_tensor("seqT", [128, NK * L], f16, kind="ExternalInput")
    wT = nc.dram_tensor("wT", [128, NK * H], f16, kind="ExternalInput")
    biasc = nc.dram_tensor("biasc", [128, NHB], f32, kind="ExternalInput")
    out = nc.dram_tensor("out", [HH, PTOT], f16, kind="ExternalOutput")

    with tile.TileContext(nc) as tc:
        with (
            tc.tile_pool(name="persist", bufs=1) as pers,
            tc.tile_pool(name="pre_ps", bufs=4, space="PSUM") as pre_ps,
            tc.tile_pool(name="stg", bufs=4) as stgp,
            tc.tile_pool(name="outp", bufs=3) as outp,
        ):
            # inputs split per k-block pair so matmuls start as pieces land
            seqT_sb = [
                pers.tile([128, 2 * L], f16, tag=f"seqT{t}", name=f"seqT{t}")
                for t in range(3)
            ]
            wT_sb = [
                pers.tile([128, 2 * H], f16, tag=f"wT{t}", name=f"wT{t}")
                for t in range(3)
            ]
            bias_sb = pers.tile([128, NHB], f32, tag="biasc")
            # split the input loads across both HWDGE rings (sync + scalar)
            # so the precompute matmuls start sooner
            for t in range(3):
                nc.sync.dma_start(wT_sb[t][:], wT[:, 2 * t * H : 2 * (t + 1) * H])
                nc.scalar.dma_start(
                    seqT_sb[t][:], seqT[:, 2 * t * L : 2 * (t + 1) * L]
                )
            nc.sync.dma_start(bias_sb[:], biasc[:])

            def w_lhsT(k, col0):  # weight block k, 128 cols at col0
                return wT_sb[k // 2][:, (k % 2) * H + col0 : (k % 2) * H + col0 + 128]

            def seq_rhs(k):
                return seqT_sb[k // 2][:, (k % 2) * L : (k % 2 + 1) * L]

            # ---- A_T[hb] = W1 @ seq^T + b, C_T[hb] = W2 @ seq^T ----
            A_sb, C_sb, C_od = [], [], []
            for hb in range(NHB):
                psA = pre_ps.tile([128, L], f32, tag="ps")
                psC = pre_ps.tile([128, L], f32, tag="ps")
                for k in range(NK):
                    nc.tensor.matmul(
                        psA[:],
                        lhsT=w_lhsT(k, hb * 128),
                        rhs=seq_rhs(k),
                        start=(k == 0),
                        stop=(k == NK - 1),
                    )
                for k in range(NK):
                    nc.tensor.matmul(
                        psC[:],
                        lhsT=w_lhsT(k, HH + hb * 128),
                        rhs=seq_rhs(k),
                        start=(k == 0),
                        stop=(k == NK - 1),
                    )
                a_t = pers.tile([128, L], f32, tag=f"A{hb}", name=f"A{hb}")
                nc.vector.tensor_scalar_add(
                    a_t[:], psA[:], bias_sb[:, hb : hb + 1]
                )
                c_t = pers.tile([128, L + 2], f16, tag=f"C{hb}", name=f"C{hb}")
                nc.vector.memset(c_t[:, L : L + 2], 0.0)
                nc.vector.tensor_copy(c_t[:, 0:L], psC[:])
                c_o = pers.tile([128, L], f16, tag=f"Co{hb}", name=f"Co{hb}")
                nc.vector.memset(c_o[:, L - 1 : L], 0.0)
                nc.vector.tensor_copy(c_o[:, 0 : L - 1], c_t[:, 1:L])
                A_sb.append(a_t)
                C_sb.append(c_t)
                C_od.append(c_o)

            def ap3(base, d1, d2):
                """Custom 3D AP [partition, d1, d2] rooted at `base`."""
                return bass_rust.AP(
                    base.tensor, base.offset, [list(base.ap[0]), list(d1), list(d2)]
                )

            # ---- main loop: 8 chunks x 3 hb blocks ----
            # chunk 0 is emitted last: it is (nearly) all ACT-fused, so the
            # kernel tail is pure ACT work that does not wait on DVE draining
            for k in list(range(1, NCHUNK)) + [0]:
                dves, merged, fused, ranges = PLAN[k]
                for hb in range(NHB):
                    ot = outp.tile([128, CH], f16, tag=f"ot{hb}")
                    if dves or merged:
                        stg = stgp.tile([128, CH], f16, tag=f"stg{hb}")
                        for cs, ln, i, s0, odd in dves:
                            src = (C_od if odd else C_sb)[hb]
                            nc.vector.tensor_scalar_add(
                                stg[:, cs : cs + ln],
                                src[:, s0 : s0 + ln],
                                A_sb[hb][:, i : i + 1],
                            )
                        for cs, ln, i, s0 in merged:
                            # rows i, i+1 (equal padded len) in one TT op
                            nc.vector.tensor_tensor(
                                out=ap3(stg[:, cs : cs + 1], [ln, 2], [1, ln]),
                                in0=ap3(C_sb[hb][:, s0 : s0 + 1], [1, 2], [1, ln]),
                                in1=ap3(A_sb[hb][:, i : i + 1], [1, 2], [0, ln]),
                                op=mybir.AluOpType.add,
                            )
                        for a, bnd in ranges:
                            nc.scalar.activation(
                                ot[:, a:bnd], stg[:, a:bnd], Tanh
                            )
                    for cs, ln, i, s0, odd in fused:
                        src = (C_od if odd else C_sb)[hb]
                        nc.scalar.activation(
                            ot[:, cs : cs + ln],
                            src[:, s0 : s0 + ln],
                            Tanh,
                            bias=A_sb[hb][:, i : i + 1],
                        )
                    nc.sync.dma_start(
                        out[hb * 128 : (hb + 1) * 128, k * CH : (k + 1) * CH],
                        ot[:],
                    )

    nc.compile()
    return nc


def _get_nc():
    if "nc" not in _CACHE:
        _CACHE["nc"] = _build_nc()
    return _CACHE["nc"]


def _pack_inputs(seq_hiddens, W, b):
    """Per-core host-side packing into the SBUF-ready layouts."""
    f16 = np.float16
    w1T = W[:, :H].T  # [k, h]
    w2T = W[:, H:].T
    in_maps = []
    for c in range(8):
        bb, hf = divmod(c, 2)
        hs = slice(hf * HH, (hf + 1) * HH)
        # seqT packed: [128, NK*L], block k cols = seq[bb].T rows k*128..
        sq = np.ascontiguousarray(
            seq_hiddens[bb].T.reshape(NK, 128, L).transpose(1, 0, 2).reshape(128, NK * L)
        ).astype(f16)
        # wT packed: [128, NK*H]; block k = [w1t_k (HH cols) | w2t_k (HH cols)]
        w1s = w1T[:, hs].reshape(NK, 128, HH)
        w2s = w2T[:, hs].reshape(NK, 128, HH)
        wpk = np.concatenate([w1s, w2s], axis=2)  # [NK, 128, 2*HH=H]
        wpk = np.ascontiguousarray(wpk.transpose(1, 0, 2).reshape(128, NK * H)).astype(f16)
        bcol = np.ascontiguousarray(b[hs].reshape(NHB, 128).T).astype(np.float32)
        in_maps.append({"seqT": sq, "wT": wpk, "biasc": bcol})
    return in_maps


def kernel(seq_hiddens, W, b):
    from concourse.bass_utils import run_bass_kernel_spmd

    seq_hiddens = np.asarray(seq_hiddens, dtype=np.float32)
    W = np.asarray(W, dtype=np.float32)
    b = np.asarray(b, dtype=np.float32)

    nc = _get_nc()
    in_maps = _pack_inputs(seq_hiddens, W, b)
    res = run_bass_kernel_spmd(nc, in_maps, list(range(8)))

    fullT = np.empty((B, H, NPAIR), np.float32)
    for c in range(8):
        bb, hf = divmod(c, 2)
        arr = res.results[c]["out"]  # [HH, PTOT] fp16
        fullT[bb, hf * HH : (hf + 1) * HH, :] = arr[:, SEL]
    return np.ascontiguousarray(fullT.transpose(0, 2, 1))


if __name__ == "__main__":
    print(f"thresh={THRESH} dve={_DVE_NS/1e3:.1f}us act={_ACT_NS/1e3:.1f}us")
    rng = np.random.RandomState(0)
    sh = rng.randn(B, L, H).astype(np.float32)
    Wv = (rng.randn(H, 2 * H) * 0.02).astype(np.float32)
    bv = np.zeros(H, np.float32)
    o = kernel(seq_hiddens=sh, W=Wv, b=bv)
    print("kernel output", o.shape, o.dtype, float(np.abs(o).max()))


# revision 6
# speedup vs baseline: 1.1914x; 1.1914x over previous
"""Handshaking kernel ('cat' type) for Trainium2, 8 NeuronCores.

Math: for each upper-triangular pair (i, j>=i):
    out[b, p(i,j), :] = tanh(W1 @ h_i + W2 @ h_j + bias),  W = [W1 | W2]

Decomposition: per-token projections in TRANSPOSED layout
    A_T[h, i] = (W1 @ seq^T)[h, i] + b[h],   C_T[h, j] = (W2 @ seq^T)[h, j]
so triangle row i of the pair dim is the contiguous slice
    out_T[:, OFF[i] : OFF[i]+L-i] = tanh(C_T[:, i:L] + A_T[:, i])
A_T[:, i] is a per-partition scalar: long rows run fused on ACT
(activation bias = A column, affine is free), short rows run as DVE
tensor_scalar adds (2x fp16 mode) + aligned ACT tanh slices; the split
is balanced by a measured cost model. Output is written as contiguous
fp16 chunk DMAs — no indirect scatter. Pair rows are padded to even
length so every DVE slice stays 4-byte aligned with even free dim.
Host gathers the padded columns out, transposes, and upcasts to f32.

Sharding: 8 cores = 4 batches x 2 halves of the hidden dim (H=768 ->
384 per core = 3 partition blocks of 128). All cores run the identical
program (SPMD).
"""

import sys
import numpy as np

for _p in ("/opt/trn_rl_repo", "/root/.axon_site/_ro/trn_rl_repo"):
    if _p not in sys.path:
        sys.path.insert(0, _p)

B, L, H = 4, 256, 768
HH = H // 2           # per-core hidden slice
NHB = HH // 128       # partition blocks per core (3)
NK = H // 128         # contraction blocks (6)
NPAIR = L * (L + 1) // 2      # 32896

# padded triangle layout: row i gets plen = len + (len odd), all offsets even
LENS = [L - i for i in range(L)]
PLENS = [l + (l & 1) for l in LENS]
POFF = np.concatenate([[0], np.cumsum(PLENS)]).astype(np.int64)
PTOT = int(POFF[-1])          # 33024
NCHUNK = 8
CH = PTOT // NCHUNK           # 4128
assert CH * NCHUNK == PTOT and CH % 2 == 0

# device column for each global pair index p (host-side unpad gather)
SEL = np.empty(NPAIR, np.int64)
for i in range(L):
    off = i * L - (i * (i - 1)) // 2
    SEL[off : off + LENS[i]] = POFF[i] + np.arange(LENS[i])


def build_segments():
    """Per chunk: list of (cs, ln, i, s0, odd) slices, ordered by cs.

    stage[:, cs:cs+ln] = C[:, j0:j0+ln] + A[:, i] with j0 = s0 (+1 if odd,
    read via the one-shifted C_od copy so the slice start stays even).
    """
    segs = [[] for _ in range(NCHUNK)]
    for i in range(L):
        start, end = int(POFF[i]), int(POFF[i]) + PLENS[i]
        s = start
        while s < end:
            k = s // CH
            e = min(end, (k + 1) * CH)
            j0 = i + (s - start)
            odd = j0 & 1
            segs[k].append((s - k * CH, e - s, i, j0 - odd, odd))
            s = e
    return segs


SEGS = build_segments()

# ---- static DVE/ACT load balance (measured-calibrated cost model, ns) ----
DVE_FIX, DVE_PER = 105.0, 0.52    # tensor_scalar (2x_1p fp16) per seg/elem
DVE_TT_PER = 1.04                 # merged tensor_tensor runs 1x
ACT_FIX, ACT_PER = 293.0, 0.833   # activation per instr/elem
MERGE_MAX = 100                   # merge 2-row pairs below this len


def _find_merges(dve_segs):
    """Pair adjacent equal-length full rows (even i, then i+1) into one
    merged TT op: returns (merged_pairs, leftover_singles)."""
    merged, singles = [], []
    by_key = {}
    for seg in dve_segs:
        cs, ln, i, s0, odd = seg
        by_key[(i, cs)] = seg
    used = set()
    for seg in dve_segs:
        cs, ln, i, s0, odd = seg
        if (i, cs) in used:
            continue
        # merge candidate: even row i, unsplit (cs..cs+ln is full row),
        # partner row i+1 at cs+ln with same ln, also unsplit
        if (
            ln <= MERGE_MAX
            and i % 2 == 0
            and not odd
            and ln == PLENS[i]
            and (i + 1, cs + ln) in by_key
            and (i + 1, cs + ln) not in used
        ):
            p = by_key[(i + 1, cs + ln)]
            if p[1] == ln and p[1] == PLENS[i + 1]:
                merged.append((cs, ln, i, s0))
                used.add((i, cs))
                used.add((i + 1, cs + ln))
                continue
        used.add((i, cs))
        singles.append(seg)
    return merged, singles


def plan_assignment():
    """Fuse rows with len >= thresh onto ACT; pick thresh to balance."""
    all_lens = sorted({ln for k in range(NCHUNK) for (_, ln, _, _, _) in SEGS[k]})
    best = None
    for thresh in all_lens + [1 << 30]:
        dve = act = 0.0
        for k in range(NCHUNK):
            cur = None
            nrange = 0
            dsegs = []
            for cs, ln, _i, _s0, _odd in SEGS[k]:
                if ln >= thresh:
                    act += ACT_FIX + ACT_PER * ln
                    cur = None
                else:
                    dsegs.append((cs, ln, _i, _s0, _odd))
                    if cur is None:
                        nrange += 1
                        cur = True
                    act += ACT_PER * ln
            mg, sg = _find_merges(dsegs)
            for _cs, ln, _i, _s0 in mg:
                dve += DVE_FIX + DVE_TT_PER * 2 * ln
            for _cs, ln, _i, _s0, _odd in sg:
                dve += DVE_FIX + DVE_PER * ln
            act += ACT_FIX * nrange
        dve *= NHB
        act *= NHB
        t = max(dve, act)
        if best is None or t < best[0]:
            best = (t, thresh, dve, act)
    _, thresh, dve, act = best
    plan = []
    for k in range(NCHUNK):
        fused, dves, ranges = [], [], []
        cur = None
        for seg in SEGS[k]:
            cs, ln = seg[0], seg[1]
            if ln >= thresh:
                fused.append(seg)
                cur = None
            else:
                dves.append(seg)
                if cur is None:
                    ranges.append([cs, cs + ln])
                    cur = ranges[-1]
                else:
                    cur[1] = cs + ln
        mg, sg = _find_merges(dves)
        plan.append((sg, mg, fused, ranges))
    return plan, thresh, dve, act


PLAN, THRESH, _DVE_NS, _ACT_NS = plan_assignment()


def _schedule_selfcheck():
    rng = np.random.RandomState(0)
    nh = 4
    Cc = rng.randn(nh, L).astype(np.float64)
    Aa = rng.randn(nh, L).astype(np.float64)
    Ce = np.concatenate([Cc, np.zeros((nh, 2))], axis=1)   # C_sb w/ pad
    Co = np.concatenate([Cc[:, 1:], np.zeros((nh, 1))], axis=1)  # C_od
    dev = np.full((nh, PTOT), np.nan)
    for k in range(NCHUNK):
        dves, merged, fused, ranges = PLAN[k]
        for cs, ln, i, s0, odd in dves + fused:
            src = Co if odd else Ce
            dev[:, k * CH + cs : k * CH + cs + ln] = (
                src[:, s0 : s0 + ln] + Aa[:, i : i + 1]
            )
        for cs, ln, i, s0 in merged:
            # row i from C[s0:s0+ln], row i+1 from C[s0+1:s0+1+ln]
            for v in range(2):
                dev[:, k * CH + cs + v * ln : k * CH + cs + (v + 1) * ln] = (
                    Ce[:, s0 + v : s0 + v + ln] + Aa[:, i + v : i + v + 1]
                )
        # every dve seg must fall inside an aligned range
        for cs, ln, _i, _s0, _odd in dves:
            assert any(a <= cs and cs + ln <= b for a, b in ranges)
        for cs, ln, _i, _s0 in merged:
            assert any(a <= cs and cs + 2 * ln <= b for a, b in ranges)
        # ranges must not overlap fused segs
        for cs, ln, _i, _s0, _odd in fused:
            assert all(b <= cs or cs + ln <= a for a, b in ranges)
    assert not np.isnan(dev).any()
    got = dev[:, SEL]
    ii, jj = np.triu_indices(L)
    exp = Aa[:, ii] + Cc[:, jj]
    assert np.allclose(got, exp), "segment schedule self-check failed"


_schedule_selfcheck()

_CACHE = {}


def _build_nc():
    import bass_rust
    import concourse.bass as bass
    import concourse.bacc as bacc
    import concourse.mybir as mybir
    import concourse.tile as tile

    f32 = mybir.dt.float32
    f16 = mybir.dt.float16
    Tanh = mybir.ActivationFunctionType.Tanh

    nc = bacc.Bacc(None, target_bir_lowering=False, debug=False)

    # host-packed inputs (see kernel() for layouts)
    seqT = nc.dram_tensor("seqT", [128, NK * L], f16, kind="ExternalInput")
    wT = nc.dram_tensor("wT", [128, NK * H], f16, kind="ExternalInput")
    biasc = nc.dram_tensor("biasc", [128, NHB], f32, kind="ExternalInput")
    out = nc.dram_tensor("out", [HH, PTOT], f16, kind="ExternalOutput")

    with tile.TileContext(nc) as tc:
        with (
            tc.tile_pool(name="persist", bufs=1) as pers,
            tc.tile_pool(name="pre_ps", bufs=4, space="PSUM") as pre_ps,
            tc.tile_pool(name="stg", bufs=4) as stgp,
            tc.tile_pool(name="outp", bufs=3) as outp,
        ):
            # inputs split per k-block pair so matmuls start as pieces land
            seqT_sb = [
                pers.tile([128, 2 * L], f16, tag=f"seqT{t}", name=f"seqT{t}")
                for t in range(3)
            ]
            wT_sb = [
                pers.tile([128, 2 * H], f16, tag=f"wT{t}", name=f"wT{t}")
                for t in range(3)
            ]
            bias_sb = pers.tile([128, NHB], f32, tag="biasc")
            # split the input loads across both HWDGE rings (sync + scalar)
            # so the precompute matmuls start sooner
            for t in range(3):
                nc.sync.dma_start(wT_sb[t][:], wT[:, 2 * t * H : 2 * (t + 1) * H])
                nc.scalar.dma_start(
                    seqT_sb[t][:], seqT[:, 2 * t * L : 2 * (t + 1) * L]
                )
            nc.sync.dma_start(bias_sb[:], biasc[:])

            def w_lhsT(k, col0):  # weight block k, 128 cols at col0
                return wT_sb[k // 2][:, (k % 2) * H + col0 : (k % 2) * H + col0 + 128]

            def seq_rhs(k):
                return seqT_sb[k // 2][:, (k % 2) * L : (k % 2 + 1) * L]

            # ---- A_T[hb] = W1 @ seq^T + b, C_T[hb] = W2 @ seq^T ----
            A_sb, C_sb, C_od = [], [], []
            for hb in range(NHB):
                psA = pre_ps.tile([128, L], f32, tag="ps")
                psC = pre_ps.tile([128, L], f32, tag="ps")
                for k in range(NK):
                    nc.tensor.matmul(
                        psA[:],
                        lhsT=w_lhsT(k, hb * 128),
                        rhs=seq_rhs(k),
                        start=(k == 0),
                        stop=(k == NK - 1),
                    )
                for k in range(NK):
                    nc.tensor.matmul(
                        psC[:],
                        lhsT=w_lhsT(k, HH + hb * 128),
                        rhs=seq_rhs(k),
                        start=(k == 0),
                        stop=(k == NK - 1),
                    )
                a_t = pers.tile([128, L], f32, tag=f"A{hb}", name=f"A{hb}")
                nc.vector.tensor_scalar_add(
                    a_t[:], psA[:], bias_sb[:, hb : hb + 1]
                )
                c_t = pers.tile([128, L + 2], f16, tag=f"C{hb}", name=f"C{hb}")
                nc.vector.memset(c_t[:, L : L + 2], 0.0)
                nc.vector.tensor_copy(c_t[:, 0:L], psC[:])
                c_o = pers.tile([128, L], f16, tag=f"Co{hb}", name=f"Co{hb}")
                nc.vector.memset(c_o[:, L - 1 : L], 0.0)
                nc.vector.tensor_copy(c_o[:, 0 : L - 1], c_t[:, 1:L])
                A_sb.append(a_t)
                C_sb.append(c_t)
                C_od.append(c_o)

            def ap3(base, d1, d2):
                """Custom 3D AP [partition, d1, d2] rooted at `base`."""
                return bass_rust.AP(
                    base.tensor, base.offset, [list(base.ap[0]), list(d1), list(d2)]
                )

            # ---- main loop: 8 chunks x 3 hb blocks ----
            for k in range(NCHUNK):
                dves, merged, fused, ranges = PLAN[k]
                for hb in range(NHB):
                    ot = outp.tile([128, CH], f16, tag=f"ot{hb}")
                    if dves or merged:
                        stg = stgp.tile([128, CH], f16, tag=f"stg{hb}")
                        for cs, ln, i, s0, odd in dves:
                            src = (C_od if odd else C_sb)[hb]
                            nc.vector.tensor_scalar_add(
                                stg[:, cs : cs + ln],
                                src[:, s0 : s0 + ln],
                                A_sb[hb][:, i : i + 1],
                            )
                        for cs, ln, i, s0 in merged:
                            # rows i, i+1 (equal padded len) in one TT op
                            nc.vector.tensor_tensor(
                                out=ap3(stg[:, cs : cs + 1], [ln, 2], [1, ln]),
                                in0=ap3(C_sb[hb][:, s0 : s0 + 1], [1, 2], [1, ln]),
                                in1=ap3(A_sb[hb][:, i : i + 1], [1, 2], [0, ln]),
                                op=mybir.AluOpType.add,
                            )
                        for a, bnd in ranges:
                            nc.scalar.activation(
                                ot[:, a:bnd], stg[:, a:bnd], Tanh
                            )
                    for cs, ln, i, s0, odd in fused:
                        src = (C_od if odd else C_sb)[hb]
                        nc.scalar.activation(
                            ot[:, cs : cs + ln],
                            src[:, s0 : s0 + ln],
                            Tanh,
                            bias=A_sb[hb][:, i : i + 1],
                        )
                    nc.sync.dma_start(
                        out[hb * 128 : (hb + 1) * 128, k * CH : (k + 1) * CH],
                        ot[:],
                    )

    nc.compile()
    return nc


def _get_nc():
    if "nc" not in _CACHE:
        _CACHE["nc"] = _build_nc()
    return _CACHE["nc"]


def _pack_inputs(seq_hiddens, W, b):
    """Per-core host-side packing into the SBUF-ready layouts."""
    f16 = np.float16
    w1T = W[:, :H].T  # [k, h]
    w2T = W[:, H:].T
    in_maps = []
    for c in range(8):
        bb, hf = divmod(c, 2)
        hs = slice(hf * HH, (hf + 1) * HH)
        # seqT packed: [128, NK*L], block k cols = seq[bb].T rows k*128..
        sq = np.ascontiguousarray(
            seq_hiddens[bb].T.reshape(NK, 128, L).transpose(1, 0, 2).reshape(128, NK * L)
        ).astype(f16)
        # wT packed: [128, NK*H]; block k = [w1t_k (HH cols) | w2t_k (HH cols)]
        w1s = w1T[:, hs].reshape(NK, 128, HH)
        w2s = w2T[:, hs].reshape(NK, 128, HH)
        wpk = np.concatenate([w1s, w2s], axis=2)  # [NK, 128, 2*HH=H]
        wpk = np.ascontiguousarray(wpk.transpose(1, 0, 2).reshape(128, NK * H)).astype(f16)
        bcol = np.ascontiguousarray(b[hs].reshape(NHB, 128).T).astype(np.float32)
        in_maps.append({"seqT": sq, "wT": wpk, "biasc": bcol})
    return in_maps


def kernel(seq_hiddens, W, b):
    from concourse.bass_utils import run_bass_kernel_spmd

    seq_hiddens = np.asarray(seq_hiddens, dtype=np.float32)
    W = np.asarray(W, dtype=np.float32)
    b = np.asarray(b, dtype=np.float32)

    nc = _get_nc()
    in_maps = _pack_inputs(seq_hiddens, W, b)
    res = run_bass_kernel_spmd(nc, in_maps, list(range(8)))

    fullT = np.empty((B, H, NPAIR), np.float32)
    for c in range(8):
        bb, hf = divmod(c, 2)
        arr = res.results[c]["out"]  # [HH, PTOT] fp16
        fullT[bb, hf * HH : (hf + 1) * HH, :] = arr[:, SEL]
    return np.ascontiguousarray(fullT.transpose(0, 2, 1))


if __name__ == "__main__":
    print(f"thresh={THRESH} dve={_DVE_NS/1e3:.1f}us act={_ACT_NS/1e3:.1f}us")
    rng = np.random.RandomState(0)
    sh = rng.randn(B, L, H).astype(np.float32)
    Wv = (rng.randn(H, 2 * H) * 0.02).astype(np.float32)
    bv = np.zeros(H, np.float32)
    o = kernel(seq_hiddens=sh, W=Wv, b=bv)
    print("kernel output", o.shape, o.dtype, float(np.abs(o).max()))


# revision 7
# speedup vs baseline: 1.1914x; 1.0000x over previous
"""Handshaking kernel ('cat' type) for Trainium2, 8 NeuronCores.

Math: for each upper-triangular pair (i, j>=i):
    out[b, p(i,j), :] = tanh(W1 @ h_i + W2 @ h_j + bias),  W = [W1 | W2]

Decomposition: per-token projections in TRANSPOSED layout
    A_T[h, i] = (W1 @ seq^T)[h, i] + b[h],   C_T[h, j] = (W2 @ seq^T)[h, j]
so triangle row i of the pair dim is the contiguous slice
    out_T[:, OFF[i] : OFF[i]+L-i] = tanh(C_T[:, i:L] + A_T[:, i])
A_T[:, i] is a per-partition scalar: long rows run fused on ACT
(activation bias = A column, affine is free), short rows run as DVE
tensor_scalar adds (2x fp16 mode) + aligned ACT tanh slices; the split
is balanced by a measured cost model. Output is written as contiguous
fp16 chunk DMAs — no indirect scatter. Pair rows are padded to even
length so every DVE slice stays 4-byte aligned with even free dim.
Host gathers the padded columns out, transposes, and upcasts to f32.

Sharding: 8 cores = 4 batches x 2 halves of the hidden dim (H=768 ->
384 per core = 3 partition blocks of 128). All cores run the identical
program (SPMD).
"""

import sys
import numpy as np

for _p in ("/opt/trn_rl_repo", "/root/.axon_site/_ro/trn_rl_repo"):
    if _p not in sys.path:
        sys.path.insert(0, _p)

B, L, H = 4, 256, 768
HH = H // 2           # per-core hidden slice
NHB = HH // 128       # partition blocks per core (3)
NK = H // 128         # contraction blocks (6)
NPAIR = L * (L + 1) // 2      # 32896

# padded triangle layout: three row classes by padded length
#   FUSED  (plen >= T1): tanh(C+bias) fused on ACT, splittable at chunk edges
#   SINGLE (T2 <= plen < T1): DVE tensor_scalar, splittable
#   BLOCKED (plen < T2): groups of even/odd pairs padded to the block max,
#       one strided 3D tensor_tensor per block (1x mode, fixed cost amortized)
LENS = [L - i for i in range(L)]
PLENS = [l + (l & 1) for l in LENS]
NCHUNK = 8
CH = 4128
MAXREAD = 2 * 8  # max rows per block bounds the C_sb zero-pad tail

# ---- measured-calibrated cost model (ns) ----
DVE_FIX, DVE_PER = 105.0, 0.52    # tensor_scalar (2x_1p fp16) per seg/elem
TT_FIX, TT_PER = 160.0, 1.04      # blocked tensor_tensor runs 1x
ACT_FIX, ACT_PER = 293.0, 0.833   # activation per instr/elem


def build_layout(t2, maxpair):
    """One-pass layout. Returns (rowoff, plain_rows, blocks, ptot)."""
    rowoff = np.zeros(L, np.int64)
    plain = []
    blocks = []  # (pos, i0, nrows, pmax)
    pos = 0
    i = 0
    while i < L:
        pl = PLENS[i]
        if pl < t2 and i % 2 == 0 and i + 1 < L:
            pmax = pl
            edge = ((pos // CH) + 1) * CH
            n = 0
            j = i
            while (
                j + 1 < L
                and j % 2 == 0
                and PLENS[j] < t2
                and n + 2 <= 2 * maxpair
                and pos + (n + 2) * pmax <= edge
            ):
                n += 2
                j += 2
            if n >= 2:
                blocks.append((pos, i, n, pmax))
                for v in range(n):
                    rowoff[i + v] = pos + v * pmax
                pos += n * pmax
                i += n
                continue
        rowoff[i] = pos
        plain.append(i)
        pos += pl
        i += 1
    return rowoff, plain, blocks, pos


def build_segments(rowoff, plain, nchunk):
    """Chunk-split slices for plain rows: (cs, ln, i, s0, odd) per chunk."""
    segs = [[] for _ in range(nchunk)]
    for i in plain:
        start, end = int(rowoff[i]), int(rowoff[i]) + PLENS[i]
        s = start
        while s < end:
            k = s // CH
            e = min(end, (k + 1) * CH)
            j0 = i + (s - start)
            odd = j0 & 1
            segs[k].append((s - k * CH, e - s, i, j0 - odd, odd))
            s = e
    return segs


def plan_all():
    best = None
    for maxpair in (4, 6, 8):
        for t2 in (80, 100, 120, 140, 160, 176, 192, 210):
            rowoff, plain, blocks, ptot = build_layout(t2, maxpair)
            nchunk = -(-ptot // CH)
            segs = build_segments(rowoff, plain, nchunk)
            blk_by_chunk = [[] for _ in range(nchunk)]
            blk_cost = 0.0
            for pos, i0, n, pmax in blocks:
                k = pos // CH
                assert pos + n * pmax <= (k + 1) * CH
                blk_by_chunk[k].append((pos - k * CH, pmax, i0, n))
                blk_cost += TT_FIX + TT_PER * n * pmax
            lens = sorted({s[1] for cs in segs for s in cs if s[1] >= t2})
            for t1 in lens + [1 << 30]:
                dve = blk_cost
                act = 0.0
                plan = []
                for k in range(nchunk):
                    items = [("b", cs, n * pmax, (cs, pmax, i0, n))
                             for cs, pmax, i0, n in blk_by_chunk[k]]
                    sgl, fus = [], []
                    for seg in segs[k]:
                        if seg[1] >= t1:
                            fus.append(seg)
                            items.append(("f", seg[0], seg[1], seg))
                            act += ACT_FIX + ACT_PER * seg[1]
                        else:
                            sgl.append(seg)
                            items.append(("s", seg[0], seg[1], seg))
                            dve += DVE_FIX + DVE_PER * seg[1]
                    items.sort(key=lambda x: x[1])
                    ranges = []
                    cur = None
                    for kind, cs, ln, _ in items:
                        if kind == "f":
                            cur = None
                            continue
                        act += ACT_PER * ln
                        if cur is None:
                            ranges.append([cs, cs + ln])
                            cur = ranges[-1]
                        else:
                            assert cur[1] == cs
                            cur[1] = cs + ln
                    act += ACT_FIX * len(ranges)
                    clen = min(CH, ptot - k * CH)
                    plan.append((clen, sgl, blk_by_chunk[k], fus, ranges))
                t = max(dve * NHB, act * NHB)
                if best is None or t < best[0]:
                    best = (t, dve * NHB, act * NHB, t1, t2, maxpair,
                            rowoff, plan, ptot, nchunk)
    return best


(_TMAX, _DVE_NS, _ACT_NS, T1, T2, MAXPAIR, ROWOFF, PLAN, PTOT,
 NCHUNK2) = plan_all()

# device column for each global pair index p (host-side unpad gather)
SEL = np.empty(NPAIR, np.int64)
for i in range(L):
    off = i * L - (i * (i - 1)) // 2
    SEL[off : off + LENS[i]] = ROWOFF[i] + np.arange(LENS[i])


def _schedule_selfcheck():
    rng = np.random.RandomState(0)
    nh = 4
    Cc = rng.randn(nh, L).astype(np.float64)
    Aa = rng.randn(nh, L).astype(np.float64)
    Ce = np.concatenate([Cc, np.zeros((nh, MAXREAD))], axis=1)   # C_sb w/ pad
    Co = np.concatenate([Cc[:, 1:], np.zeros((nh, 1))], axis=1)  # C_od
    dev = np.full((nh, PTOT), np.nan)
    for k in range(NCHUNK2):
        clen, sgl, blks, fused, ranges = PLAN[k]
        for cs, ln, i, s0, odd in sgl + fused:
            src = Co if odd else Ce
            dev[:, k * CH + cs : k * CH + cs + ln] = (
                src[:, s0 : s0 + ln] + Aa[:, i : i + 1]
            )
        for cs, pmax, i0, n in blks:
            # row i0+v covers [cs+v*pmax, cs+(v+1)*pmax) from C[i0+v ...]
            for v in range(n):
                dev[:, k * CH + cs + v * pmax : k * CH + cs + (v + 1) * pmax] = (
                    Ce[:, i0 + v : i0 + v + pmax] + Aa[:, i0 + v : i0 + v + 1]
                )
            assert cs + n * pmax <= clen
        # every dve item must fall inside an aligned range
        for cs, ln, _i, _s0, _odd in sgl:
            assert any(a <= cs and cs + ln <= b for a, b in ranges)
        for cs, pmax, _i0, n in blks:
            assert any(a <= cs and cs + n * pmax <= b for a, b in ranges)
        # ranges must not overlap fused segs
        for cs, ln, _i, _s0, _odd in fused:
            assert all(b <= cs or cs + ln <= a for a, b in ranges)
    assert not np.isnan(dev).any()
    got = dev[:, SEL]
    ii, jj = np.triu_indices(L)
    exp = Aa[:, ii] + Cc[:, jj]
    assert np.allclose(got, exp), "segment schedule self-check failed"


_schedule_selfcheck()

_CACHE = {}


def _build_nc():
    import bass_rust
    import concourse.bass as bass
    import concourse.bacc as bacc
    import concourse.mybir as mybir
    import concourse.tile as tile

    f32 = mybir.dt.float32
    f16 = mybir.dt.float16
    Tanh = mybir.ActivationFunctionType.Tanh

    nc = bacc.Bacc(None, target_bir_lowering=False, debug=False)

    # host-packed inputs (see kernel() for layouts)
    seqT = nc.dram_tensor("seqT", [128, NK * L], f16, kind="ExternalInput")
    wT = nc.dram_tensor("wT", [128, NK * H], f16, kind="ExternalInput")
    biasc = nc.dram_tensor("biasc", [128, NHB], f32, kind="ExternalInput")
    out = nc.dram_tensor("out", [HH, PTOT], f16, kind="ExternalOutput")

    with tile.TileContext(nc) as tc:
        with (
            tc.tile_pool(name="persist", bufs=1) as pers,
            tc.tile_pool(name="pre_ps", bufs=4, space="PSUM") as pre_ps,
            tc.tile_pool(name="stg", bufs=4) as stgp,
            tc.tile_pool(name="outp", bufs=3) as outp,
        ):
            # inputs split per k-block pair so matmuls start as pieces land
            seqT_sb = [
                pers.tile([128, 2 * L], f16, tag=f"seqT{t}", name=f"seqT{t}")
                for t in range(3)
            ]
            wT_sb = [
                pers.tile([128, 2 * H], f16, tag=f"wT{t}", name=f"wT{t}")
                for t in range(3)
            ]
            bias_sb = pers.tile([128, NHB], f32, tag="biasc")
            # split the input loads across both HWDGE rings (sync + scalar)
            # so the precompute matmuls start sooner
            for t in range(3):
                nc.sync.dma_start(wT_sb[t][:], wT[:, 2 * t * H : 2 * (t + 1) * H])
                nc.scalar.dma_start(
                    seqT_sb[t][:], seqT[:, 2 * t * L : 2 * (t + 1) * L]
                )
            nc.sync.dma_start(bias_sb[:], biasc[:])

            def w_lhsT(k, col0):  # weight block k, 128 cols at col0
                return wT_sb[k // 2][:, (k % 2) * H + col0 : (k % 2) * H + col0 + 128]

            def seq_rhs(k):
                return seqT_sb[k // 2][:, (k % 2) * L : (k % 2 + 1) * L]

            # ---- A_T[hb] = W1 @ seq^T + b, C_T[hb] = W2 @ seq^T ----
            A_sb, C_sb, C_od = [], [], []
            for hb in range(NHB):
                psA = pre_ps.tile([128, L], f32, tag="ps")
                psC = pre_ps.tile([128, L], f32, tag="ps")
                for k in range(NK):
                    nc.tensor.matmul(
                        psA[:],
                        lhsT=w_lhsT(k, hb * 128),
                        rhs=seq_rhs(k),
                        start=(k == 0),
                        stop=(k == NK - 1),
                    )
                for k in range(NK):
                    nc.tensor.matmul(
                        psC[:],
                        lhsT=w_lhsT(k, HH + hb * 128),
                        rhs=seq_rhs(k),
                        start=(k == 0),
                        stop=(k == NK - 1),
                    )
                a_t = pers.tile([128, L], f32, tag=f"A{hb}", name=f"A{hb}")
                nc.vector.tensor_scalar_add(
                    a_t[:], psA[:], bias_sb[:, hb : hb + 1]
                )
                c_t = pers.tile([128, L + MAXREAD], f16, tag=f"C{hb}", name=f"C{hb}")
                nc.vector.memset(c_t[:, L : L + MAXREAD], 0.0)
                nc.vector.tensor_copy(c_t[:, 0:L], psC[:])
                c_o = pers.tile([128, L], f16, tag=f"Co{hb}", name=f"Co{hb}")
                nc.vector.memset(c_o[:, L - 1 : L], 0.0)
                nc.vector.tensor_copy(c_o[:, 0 : L - 1], c_t[:, 1:L])
                A_sb.append(a_t)
                C_sb.append(c_t)
                C_od.append(c_o)

            def ap3(base, d1, d2):
                """Custom 3D AP [partition, d1, d2] rooted at `base`."""
                return bass_rust.AP(
                    base.tensor, base.offset, [list(base.ap[0]), list(d1), list(d2)]
                )

            # ---- main loop: chunks x 3 hb blocks ----
            for k in range(NCHUNK2):
                clen, sgl, blks, fused, ranges = PLAN[k]
                for hb in range(NHB):
                    ot = outp.tile([128, CH], f16, tag=f"ot{hb}")
                    if sgl or blks:
                        stg = stgp.tile([128, CH], f16, tag=f"stg{hb}")
                        for cs, ln, i, s0, odd in sgl:
                            src = (C_od if odd else C_sb)[hb]
                            nc.vector.tensor_scalar_add(
                                stg[:, cs : cs + ln],
                                src[:, s0 : s0 + ln],
                                A_sb[hb][:, i : i + 1],
                            )
                        for cs, pmax, i0, n in blks:
                            # rows i0..i0+n-1 padded to pmax in one TT op
                            nc.vector.tensor_tensor(
                                out=ap3(stg[:, cs : cs + 1], [pmax, n], [1, pmax]),
                                in0=ap3(C_sb[hb][:, i0 : i0 + 1], [1, n], [1, pmax]),
                                in1=ap3(A_sb[hb][:, i0 : i0 + 1], [1, n], [0, pmax]),
                                op=mybir.AluOpType.add,
                            )
                        for a, bnd in ranges:
                            nc.scalar.activation(
                                ot[:, a:bnd], stg[:, a:bnd], Tanh
                            )
                    for cs, ln, i, s0, odd in fused:
                        src = (C_od if odd else C_sb)[hb]
                        nc.scalar.activation(
                            ot[:, cs : cs + ln],
                            src[:, s0 : s0 + ln],
                            Tanh,
                            bias=A_sb[hb][:, i : i + 1],
                        )
                    nc.sync.dma_start(
                        out[hb * 128 : (hb + 1) * 128, k * CH : k * CH + clen],
                        ot[:, 0:clen],
                    )

    nc.compile()
    return nc


def _get_nc():
    if "nc" not in _CACHE:
        _CACHE["nc"] = _build_nc()
    return _CACHE["nc"]


def _pack_inputs(seq_hiddens, W, b):
    """Per-core host-side packing into the SBUF-ready layouts."""
    f16 = np.float16
    w1T = W[:, :H].T  # [k, h]
    w2T = W[:, H:].T
    in_maps = []
    for c in range(8):
        bb, hf = divmod(c, 2)
        hs = slice(hf * HH, (hf + 1) * HH)
        # seqT packed: [128, NK*L], block k cols = seq[bb].T rows k*128..
        sq = np.ascontiguousarray(
            seq_hiddens[bb].T.reshape(NK, 128, L).transpose(1, 0, 2).reshape(128, NK * L)
        ).astype(f16)
        # wT packed: [128, NK*H]; block k = [w1t_k (HH cols) | w2t_k (HH cols)]
        w1s = w1T[:, hs].reshape(NK, 128, HH)
        w2s = w2T[:, hs].reshape(NK, 128, HH)
        wpk = np.concatenate([w1s, w2s], axis=2)  # [NK, 128, 2*HH=H]
        wpk = np.ascontiguousarray(wpk.transpose(1, 0, 2).reshape(128, NK * H)).astype(f16)
        bcol = np.ascontiguousarray(b[hs].reshape(NHB, 128).T).astype(np.float32)
        in_maps.append({"seqT": sq, "wT": wpk, "biasc": bcol})
    return in_maps


def kernel(seq_hiddens, W, b):
    from concourse.bass_utils import run_bass_kernel_spmd

    seq_hiddens = np.asarray(seq_hiddens, dtype=np.float32)
    W = np.asarray(W, dtype=np.float32)
    b = np.asarray(b, dtype=np.float32)

    nc = _get_nc()
    in_maps = _pack_inputs(seq_hiddens, W, b)
    res = run_bass_kernel_spmd(nc, in_maps, list(range(8)))

    fullT = np.empty((B, H, NPAIR), np.float32)
    for c in range(8):
        bb, hf = divmod(c, 2)
        arr = res.results[c]["out"]  # [HH, PTOT] fp16
        fullT[bb, hf * HH : (hf + 1) * HH, :] = arr[:, SEL]
    return np.ascontiguousarray(fullT.transpose(0, 2, 1))


if __name__ == "__main__":
    print(f"thresh={THRESH} dve={_DVE_NS/1e3:.1f}us act={_ACT_NS/1e3:.1f}us")
    rng = np.random.RandomState(0)
    sh = rng.randn(B, L, H).astype(np.float32)
    Wv = (rng.randn(H, 2 * H) * 0.02).astype(np.float32)
    bv = np.zeros(H, np.float32)
    o = kernel(seq_hiddens=sh, W=Wv, b=bv)
    print("kernel output", o.shape, o.dtype, float(np.abs(o).max()))
